# revision 41
# baseline (speedup 1.0000x reference)
"""YOLOv3-style detection decode on 8 Trainium2 NeuronCores (pure batch data-parallel).

Contract: kernel(**inputs) takes the FULL inputs from setup_inputs() and returns
the FULL output of reference(). Batch dim 32 is sharded 4-per-core across 8
cores. Host ships only the 15 used channels, pre-packed into the output's AoS
row order; the device does the decode math (threshold mask, grid add, exp,
anchor scaling, batch-index fill).

Engine split (single chunk):
  GpSimd : P col0 := batch index b per section (4 memsets at entry, no DMA)
  ACT    : exp(w,h) into P cols 3:5
  DVE    : V0 m = (conf > thr) -> f16 mask [128,336]
           V1 grid-add (x,y f32 + A2) into P cols 1:3
           V2 P *= S5 (per-row scales (1,t,t,aw,ah), f16 2x mode)
           V3 P *= m broadcast 5-wide, in 4 quarters; each quarter's output
              DMA is issued as soon as it is ready (rings alternate)
Precision: x,y ride f32 (cancellation in col+x rules out f16); w,h ride f16
(exp output err ~ exponent abs err ~5e-4); conf rides f16 with a host-side
comparison-preserving rounding (elements whose f16 rounding would flip
`conf > thr` are nudged one ulp back, so the device f16 compare equals the
reference f32 compare exactly); output is f16 (max |box| ~27k < f16 max,
rel err ~5e-4 << 2e-2 tolerance).
"""
import sys

sys.path.insert(0, "/opt/trn_rl_repo")

import numpy as np

N_CORES = 8
B_TOTAL = 32
B_PER_CORE = B_TOTAL // N_CORES
IMG = 416.0

ANCHORS = {
    13: np.array([[116.0, 90.0], [156.0, 198.0], [373.0, 326.0]], np.float32),
    26: np.array([[30.0, 61.0], [62.0, 45.0], [59.0, 119.0]], np.float32),
    52: np.array([[10.0, 13.0], [16.0, 30.0], [33.0, 23.0]], np.float32),
}
HEAD_ORDER = [13, 26, 52]
ROWS_VALID = sum(3 * h * h for h in HEAD_ORDER)   # 10647
Q = 84                                            # row-slots per partition
ROWS_PAD = 128 * Q                                # 10752
S = B_PER_CORE                                    # 4 sections per core

# din32: [xy(672)]  f32
C_XY = 0
W32 = 2 * S * Q                                   # 672
# din16: [conf(336) | wh(672) | A2(168) | S5(420) | bvals(4) | negthr-f32(2)]
C_WH = S * Q                                      # 336
C_A2 = C_WH + 2 * S * Q                           # 1008
C_S5 = C_A2 + 2 * Q                               # 1176
C_BV = C_S5 + 5 * Q                               # 1596
W16 = C_BV + S + 2                                # 1602
W_OUT = 5 * S * Q                                 # 1680


def _build_constants():
    """Per-row-slot constants, indexed by flat row r = p*Q + q within a
    section: A2 (col,row grid offsets), S5 (1,t,t,aw,ah). Pad rows get 0."""
    r = np.arange(ROWS_PAD)
    a2 = np.zeros((ROWS_PAD, 2), np.float32)
    s5 = np.zeros((ROWS_PAD, 5), np.float32)
    off = 0
    for h in HEAD_ORDER:
        n = 3 * h * h
        lo, hi = off, off + n
        l = r[lo:hi] - off
        pos = l // 3
        anc = l % 3
        t = IMG / h
        a2[lo:hi, 0] = pos % h
        a2[lo:hi, 1] = pos // h
        s5[lo:hi, 0] = 1.0
        s5[lo:hi, 1] = t
        s5[lo:hi, 2] = t
        s5[lo:hi, 3] = ANCHORS[h][anc, 0]
        s5[lo:hi, 4] = ANCHORS[h][anc, 1]
        off = hi
    a2 = a2.reshape(128, Q, 2).reshape(128, 2 * Q)
    s5 = s5.reshape(128, Q, 5).reshape(128, 5 * Q)
    return a2.astype(np.float16), s5.astype(np.float16)


_A2, _S5 = _build_constants()

_STATE = None


def _build_program():
    """Raw Bacc program with manual semaphores (one chunk, V3/output in
    quarters so output DMA drains while DVE still computes)."""
    import concourse.bass as bass
    import concourse.bacc as bacc
    from concourse import mybir

    # Skip the Bass-constructor all-engine barrier (~0.8us).
    _orig_barrier = bass.Bass.all_engine_barrier
    bass.Bass.all_engine_barrier = lambda self, *a, **k: None
    try:
        nc = bacc.Bacc("TRN2", target_bir_lowering=False, debug=False)
    finally:
        bass.Bass.all_engine_barrier = _orig_barrier
    f32 = mybir.dt.float32
    f16 = mybir.dt.float16
    op = mybir.AluOpType

    IN32 = nc.dram_tensor("din32", [128, W32], f32, kind="ExternalInput")
    IN16 = nc.dram_tensor("din16", [128, W16], f16, kind="ExternalInput")
    OUT = nc.dram_tensor("dout", [128, W_OUT], f16, kind="ExternalOutput")

    t32 = nc.alloc_sbuf_tensor("t32", [128, W32], f32)
    t16 = nc.alloc_sbuf_tensor("t16", [128, W16], f16)
    tP = nc.alloc_sbuf_tensor("tp", [128, W_OUT], f16)
    tM5 = nc.alloc_sbuf_tensor("tm5", [128, W_OUT], f16)

    s_cs = nc.alloc_semaphore("s_cs")   # A2+S5+bvals DMA
    s_x1 = nc.alloc_semaphore("s_x1")   # xy sections 0-1 DMA
    s_x2 = nc.alloc_semaphore("s_x2")   # xy sections 2-3 DMA
    s_w = nc.alloc_semaphore("s_w")     # wh DMA
    s_cf = nc.alloc_semaphore("s_cf")   # conf DMA
    s_act = nc.alloc_semaphore("s_act")  # ACT exp(+1), M5a(+1), M5b(+1)
    s_v = nc.alloc_semaphore("s_v")     # DVE chain
    s_o = nc.alloc_semaphore("s_o")     # output DMAs

    negthr = t16.ap()[:, W16 - 2:W16].bitcast(f32)
    bvals = (
        t16.ap()[:, C_BV:C_BV + S].unsqueeze(-1).broadcast_to((128, S, Q))
    )
    xy = t32.ap().rearrange("p (s t c) -> p s t c", s=S, c=2)
    conf = t16.ap()[:, 0:C_WH]
    wh = t16.ap()[:, C_WH:C_A2].rearrange("p (s t c) -> p s t c", s=S, c=2)
    a2 = (
        t16.ap()[:, C_A2:C_S5]
        .rearrange("p (t c) -> p t c", c=2)
        .unsqueeze(1)
        .broadcast_to((128, S, Q, 2))
    )
    s5 = t16.ap()[:, C_S5:C_BV].unsqueeze(1).broadcast_to((128, S, 5 * Q))
    P = tP.ap().rearrange("p (s t c) -> p s t c", s=S, c=5)
    Pf = tP.ap().rearrange("p (s f) -> p s f", s=S)

    # --- input DMAs; global land order ~ [wh, A2S5, conf, xy1, xy2]
    nc.sync.dma_start(t16.ap()[:, C_A2:], IN16.ap()[:, C_A2:]).then_inc(s_cs, 16)
    MIDXY = S * Q
    nc.sync.dma_start(t32.ap()[:, :MIDXY], IN32.ap()[:, :MIDXY]).then_inc(s_x1, 16)
    nc.sync.dma_start(t32.ap()[:, MIDXY:], IN32.ap()[:, MIDXY:]).then_inc(s_x2, 16)
    nc.scalar.dma_start(
        t16.ap()[:, C_WH:C_A2], IN16.ap()[:, C_WH:C_A2]
    ).then_inc(s_w, 16)
    nc.scalar.dma_start(t16.ap()[:, :C_WH], IN16.ap()[:, :C_WH]).then_inc(s_cf, 16)

    # --- ACT: exp into P cols 3:5, then M5 = sign(conf - thr) 5-wide.
    # sign lives in the same act table as exp (no table swap); the relu that
    # turns {-1,0,1} into {0,0,1} is fused into the DVE mask-multiply below.
    nc.scalar.wait_ge(s_w, 16)
    nc.scalar.activation(
        P[:, :, :, 3:5], wh, mybir.ActivationFunctionType.Exp, bias=0.0
    ).then_inc(s_act, 1)
    m5 = tM5.ap().rearrange("p (t c) -> p t c", c=5)
    cb = conf.unsqueeze(-1).broadcast_to((128, S * Q, 5))
    HQ = S * Q // 2
    nc.scalar.wait_ge(s_cf, 16)
    nc.scalar.wait_ge(s_cs, 16)  # negthr bits
    nc.scalar.activation(
        m5[:, :HQ], cb[:, :HQ], mybir.ActivationFunctionType.Sign, bias=negthr
    ).then_inc(s_act, 1)
    nc.scalar.activation(
        m5[:, HQ:], cb[:, HQ:], mybir.ActivationFunctionType.Sign, bias=negthr
    ).then_inc(s_act, 1)

    # --- DVE: b-fill, then a 2-section pipeline V1/V2/V3 per half so the
    # first output half departs before the xy2 DMA has even landed
    H_OUT = W_OUT // 2
    nc.vector.wait_ge(s_cs, 16)
    nc.vector.tensor_copy(out=P[:, :, :, 0], in_=bvals).then_inc(s_v, 1)
    # half 1 (sections 0-1): V1a, V2h1, V3h1
    nc.vector.wait_ge(s_x1, 16)
    nc.vector.tensor_tensor(
        P[:, :2, :, 1:3], xy[:, :2], a2[:, :2], op.add
    ).then_inc(s_v, 1)
    nc.vector.wait_ge(s_v, 2)
    nc.vector.wait_ge(s_act, 1)
    nc.vector.tensor_tensor(
        Pf[:, :2], Pf[:, :2], s5[:, :2], op.mult
    ).then_inc(s_v, 1)
    nc.vector.wait_ge(s_act, 2)
    nc.vector.tensor_scalar(
        tM5.ap()[:, :H_OUT], tM5.ap()[:, :H_OUT], 0.0, None, op.max
    ).then_inc(s_v, 1)
    nc.vector.wait_ge(s_v, 4)
    nc.vector.tensor_tensor(
        tP.ap()[:, :H_OUT], tP.ap()[:, :H_OUT], tM5.ap()[:, :H_OUT], op.mult
    ).then_inc(s_v, 1)
    # half 2 (sections 2-3): V1b, V2h2, V3h2
    nc.vector.wait_ge(s_x2, 16)
    nc.vector.tensor_tensor(
        P[:, 2:, :, 1:3], xy[:, 2:], a2[:, 2:], op.add
    ).then_inc(s_v, 1)
    nc.vector.wait_ge(s_v, 6)
    nc.vector.tensor_tensor(
        Pf[:, 2:], Pf[:, 2:], s5[:, 2:], op.mult
    ).then_inc(s_v, 1)
    nc.vector.wait_ge(s_act, 3)
    nc.vector.tensor_scalar(
        tM5.ap()[:, H_OUT:], tM5.ap()[:, H_OUT:], 0.0, None, op.max
    ).then_inc(s_v, 1)
    nc.vector.wait_ge(s_v, 8)
    nc.vector.tensor_tensor(
        tP.ap()[:, H_OUT:], tP.ap()[:, H_OUT:], tM5.ap()[:, H_OUT:], op.mult
    ).then_inc(s_v, 1)

    # --- output DMAs (one per half, both rings). The exit wait below rides
    # on the DVE chain: the last DMA's data lands ~1us into the ~7us NEFF
    # exit ritual, far before the runtime reads the output.
    nc.sync.wait_ge(s_v, 5)
    nc.sync.dma_start(OUT.ap()[:, :H_OUT], tP.ap()[:, :H_OUT]).then_inc(s_o, 16)
    nc.scalar.wait_ge(s_v, 9)
    nc.scalar.dma_start(OUT.ap()[:, H_OUT:], tP.ap()[:, H_OUT:]).then_inc(s_o, 16)

    # PE joins the exit barrier once compute retires
    nc.tensor.wait_ge(s_v, 9)
    nc.compile()
    return nc


def _pack_rows(heads_np):
    """Full head tensors -> [32, 128, Q, 5] padded AoS rows (pos, anchor, ch),
    heads concatenated in HEAD_ORDER."""
    blocks = []
    for h in HEAD_ORDER:
        arr = heads_np[h]
        hw = h * h
        sel = arr.reshape(B_TOTAL, 3, 85, hw)[:, :, 0:5, :]
        blocks.append(sel.transpose(0, 3, 1, 2).reshape(B_TOTAL, hw * 3, 5))
    rows = np.concatenate(blocks, axis=1)
    pad = np.zeros((B_TOTAL, ROWS_PAD - ROWS_VALID, 5), np.float32)
    rows = np.concatenate([rows, pad], axis=1)
    return rows.reshape(B_TOTAL, 128, Q, 5)


def _conf_f16_preserving(conf32, thr):
    """f16-encode conf so the device compare (f16 conf > f16 thr) equals the
    reference f32 compare elementwise: nudge any element whose rounding
    flipped the compare one ulp back across thr."""
    c16 = conf32.astype(np.float16)
    want = conf32 > thr
    for _ in range(3):
        got = c16.astype(np.float32) > thr
        bad = got != want
        if not bad.any():
            break
        target = np.where(want[bad], np.float16(np.inf), np.float16(-np.inf))
        c16[bad] = np.nextafter(c16[bad], target)
    return c16


def kernel(output_13, output_26, output_52, thresh):
    global _STATE
    if _STATE is None:
        _STATE = _build_program()
    nc = _STATE

    from concourse.bass_utils import run_bass_kernel_spmd

    heads_np = {13: np.asarray(output_13, np.float32),
                26: np.asarray(output_26, np.float32),
                52: np.asarray(output_52, np.float32)}
    thr = float(np.asarray(thresh))

    R = _pack_rows(heads_np)   # [32, 128, Q, 5]

    in_maps = []
    for core in range(N_CORES):
        sub = R[core * S:(core + 1) * S]          # [S, 128, Q, 5]
        din32 = sub[..., 1:3].transpose(1, 0, 2, 3).reshape(128, 2 * S * Q).copy()
        din16 = np.empty((128, W16), np.float16)
        din16[:, 0:C_WH] = _conf_f16_preserving(
            sub[..., 0].transpose(1, 0, 2).reshape(128, S * Q), thr
        )
        din16[:, C_WH:C_A2] = (
            sub[..., 3:5].transpose(1, 0, 2, 3).reshape(128, 2 * S * Q)
        )
        din16[:, C_A2:C_S5] = _A2
        din16[:, C_S5:C_BV] = _S5
        din16[:, C_BV:C_BV + S] = (core * S + np.arange(S, dtype=np.float32))[None, :]
        negthr_bits = np.array([-thr], np.float32).view(np.uint16)
        din16.view(np.uint16)[:, W16 - 2:W16] = negthr_bits[None, :]
        in_maps.append({"din32": din32, "din16": din16})

    res = run_bass_kernel_spmd(nc, in_maps, core_ids=list(range(N_CORES)))

    # Unshard: per core, per section -> head blocks -> global row order.
    out = np.empty((ROWS_VALID * B_TOTAL, 5), np.float32)
    head_rows = {h: 3 * h * h for h in HEAD_ORDER}
    head_off = {}
    acc = 0
    for h in HEAD_ORDER:
        head_off[h] = acc
        acc += head_rows[h] * B_TOTAL
    for core in range(N_CORES):
        o = res.results[core]["dout"]             # [128, 1680] f16
        for s in range(S):
            bg = core * S + s
            sec = o[:, s * 5 * Q:(s + 1) * 5 * Q].astype(np.float32)
            rows = sec.reshape(128 * Q, 5)[:ROWS_VALID]
            off = 0
            for h in HEAD_ORDER:
                n = head_rows[h]
                dst = head_off[h] + bg * n
                out[dst:dst + n] = rows[off:off + n]
                off += n
    return out


# revision 45
# speedup vs baseline: 1.0324x; 1.0324x over previous
"""YOLOv3-style detection decode on 8 Trainium2 NeuronCores (pure batch data-parallel).

Contract: kernel(**inputs) takes the FULL inputs from setup_inputs() and returns
the FULL output of reference(). Batch dim 32 is sharded 4-per-core across 8
cores. Host ships only the 15 used channels, pre-packed into the output's AoS
row order; the device does the decode math (threshold mask, grid add, exp,
anchor scaling, batch-index fill).

Engine split (single chunk):
  GpSimd : P col0 := batch index b per section (4 memsets at entry, no DMA)
  ACT    : exp(w,h) into P cols 3:5
  DVE    : V0 m = (conf > thr) -> f16 mask [128,336]
           V1 grid-add (x,y f32 + A2) into P cols 1:3
           V2 P *= S5 (per-row scales (1,t,t,aw,ah), f16 2x mode)
           V3 P *= m broadcast 5-wide, in 4 quarters; each quarter's output
              DMA is issued as soon as it is ready (rings alternate)
Precision: x,y ride f32 (cancellation in col+x rules out f16); w,h ride f16
(exp output err ~ exponent abs err ~5e-4); conf rides f16 with a host-side
comparison-preserving rounding (elements whose f16 rounding would flip
`conf > thr` are nudged one ulp back, so the device f16 compare equals the
reference f32 compare exactly); output is f16 (max |box| ~27k < f16 max,
rel err ~5e-4 << 2e-2 tolerance).
"""
import sys

sys.path.insert(0, "/opt/trn_rl_repo")

import numpy as np

N_CORES = 8
B_TOTAL = 32
B_PER_CORE = B_TOTAL // N_CORES
IMG = 416.0

ANCHORS = {
    13: np.array([[116.0, 90.0], [156.0, 198.0], [373.0, 326.0]], np.float32),
    26: np.array([[30.0, 61.0], [62.0, 45.0], [59.0, 119.0]], np.float32),
    52: np.array([[10.0, 13.0], [16.0, 30.0], [33.0, 23.0]], np.float32),
}
HEAD_ORDER = [13, 26, 52]
ROWS_VALID = sum(3 * h * h for h in HEAD_ORDER)   # 10647
Q = 84                                            # row-slots per partition
ROWS_PAD = 128 * Q                                # 10752
S = B_PER_CORE                                    # 4 sections per core

# din32: [xy(672)]  f32
C_XY = 0
W32 = 2 * S * Q                                   # 672
# din16: [conf(336) | wh(672) | A2(168) | S5(420) | bvals(4) | negthr-f32(2)]
C_WH = S * Q                                      # 336
C_A2 = C_WH + 2 * S * Q                           # 1008
C_S5 = C_A2 + 2 * Q                               # 1176
C_BV = C_S5 + 5 * Q                               # 1596
W16 = C_BV + S + 2                                # 1602
W_OUT = 5 * S * Q                                 # 1680


def _build_constants():
    """Per-row-slot constants, indexed by flat row r = p*Q + q within a
    section: A2 (col,row grid offsets), S5 (1,t,t,aw,ah). Pad rows get 0."""
    r = np.arange(ROWS_PAD)
    a2 = np.zeros((ROWS_PAD, 2), np.float32)
    s5 = np.zeros((ROWS_PAD, 5), np.float32)
    off = 0
    for h in HEAD_ORDER:
        n = 3 * h * h
        lo, hi = off, off + n
        l = r[lo:hi] - off
        pos = l // 3
        anc = l % 3
        t = IMG / h
        a2[lo:hi, 0] = pos % h
        a2[lo:hi, 1] = pos // h
        s5[lo:hi, 0] = 1.0
        s5[lo:hi, 1] = t
        s5[lo:hi, 2] = t
        s5[lo:hi, 3] = ANCHORS[h][anc, 0]
        s5[lo:hi, 4] = ANCHORS[h][anc, 1]
        off = hi
    a2 = a2.reshape(128, Q, 2).reshape(128, 2 * Q)
    s5 = s5.reshape(128, Q, 5).reshape(128, 5 * Q)
    return a2.astype(np.float16), s5.astype(np.float16)


_A2, _S5 = _build_constants()

_STATE = None


def _build_program():
    """Raw Bacc program with manual semaphores (one chunk, V3/output in
    quarters so output DMA drains while DVE still computes)."""
    import concourse.bass as bass
    import concourse.bacc as bacc
    from concourse import mybir

    # Skip the Bass-constructor all-engine barrier (~0.8us).
    _orig_barrier = bass.Bass.all_engine_barrier
    bass.Bass.all_engine_barrier = lambda self, *a, **k: None
    try:
        nc = bacc.Bacc("TRN2", target_bir_lowering=False, debug=False)
    finally:
        bass.Bass.all_engine_barrier = _orig_barrier
    f32 = mybir.dt.float32
    f16 = mybir.dt.float16
    op = mybir.AluOpType

    IN32 = nc.dram_tensor("din32", [128, W32], f32, kind="ExternalInput")
    IN16 = nc.dram_tensor("din16", [128, W16], f16, kind="ExternalInput")
    OUT = nc.dram_tensor("dout", [128, W_OUT], f16, kind="ExternalOutput")

    t32 = nc.alloc_sbuf_tensor("t32", [128, W32], f32)
    t16 = nc.alloc_sbuf_tensor("t16", [128, W16], f16)
    tP = nc.alloc_sbuf_tensor("tp", [128, W_OUT], f16)
    tM = nc.alloc_sbuf_tensor("tm", [128, S * Q], f16)
    tM5 = nc.alloc_sbuf_tensor("tm5", [128, W_OUT], f16)

    s_cs = nc.alloc_semaphore("s_cs")   # A2+S5+bvals DMA
    s_x1 = nc.alloc_semaphore("s_x1")   # xy sections 0-1 DMA
    s_x2 = nc.alloc_semaphore("s_x2")   # xy sections 2-3 DMA
    s_w = nc.alloc_semaphore("s_w")     # wh DMA
    s_cf = nc.alloc_semaphore("s_cf")   # conf DMA
    s_act = nc.alloc_semaphore("s_act")  # ACT exp(+1), M5a(+1), M5b(+1)
    s_v = nc.alloc_semaphore("s_v")     # DVE chain
    s_o = nc.alloc_semaphore("s_o")     # output DMAs

    thr = t16.ap()[:, W16 - 2:W16].bitcast(f32)
    bvals = (
        t16.ap()[:, C_BV:C_BV + S].unsqueeze(-1).broadcast_to((128, S, Q))
    )
    xy = t32.ap().rearrange("p (s t c) -> p s t c", s=S, c=2)
    conf = t16.ap()[:, 0:C_WH]
    wh = t16.ap()[:, C_WH:C_A2].rearrange("p (s t c) -> p s t c", s=S, c=2)
    a2 = (
        t16.ap()[:, C_A2:C_S5]
        .rearrange("p (t c) -> p t c", c=2)
        .unsqueeze(1)
        .broadcast_to((128, S, Q, 2))
    )
    s5 = t16.ap()[:, C_S5:C_BV].unsqueeze(1).broadcast_to((128, S, 5 * Q))
    P = tP.ap().rearrange("p (s t c) -> p s t c", s=S, c=5)
    Pf = tP.ap().rearrange("p (s f) -> p s f", s=S)

    # --- input DMAs; global land order ~ [wh, A2S5, conf, xy1, xy2]
    nc.sync.dma_start(t16.ap()[:, C_A2:], IN16.ap()[:, C_A2:]).then_inc(s_cs, 16)
    MIDXY = S * Q
    nc.sync.dma_start(t32.ap()[:, :MIDXY], IN32.ap()[:, :MIDXY]).then_inc(s_x1, 16)
    nc.sync.dma_start(t32.ap()[:, MIDXY:], IN32.ap()[:, MIDXY:]).then_inc(s_x2, 16)
    nc.scalar.dma_start(
        t16.ap()[:, C_WH:C_A2], IN16.ap()[:, C_WH:C_A2]
    ).then_inc(s_w, 16)
    nc.scalar.dma_start(t16.ap()[:, :C_WH], IN16.ap()[:, :C_WH]).then_inc(s_cf, 16)

    # --- ACT: exp into P cols 3:5, b-fill of P col0, then the contiguous
    # 5-wide mask M5 (copy of the DVE-computed m, broadcast per row)
    nc.scalar.wait_ge(s_w, 16)
    nc.scalar.activation(
        P[:, :, :, 3:5], wh, mybir.ActivationFunctionType.Exp, bias=0.0
    ).then_inc(s_act, 1)
    nc.scalar.wait_ge(s_cs, 16)
    nc.scalar.activation(
        P[:, :, :, 0], bvals, mybir.ActivationFunctionType.Copy
    ).then_inc(s_act, 1)
    m5 = tM5.ap().rearrange("p (t c) -> p t c", c=5)
    mb = tM.ap().unsqueeze(-1).broadcast_to((128, S * Q, 5))
    HQ = S * Q // 2
    nc.scalar.wait_ge(s_v, 1)  # V0 done
    nc.scalar.activation(
        m5[:, :HQ], mb[:, :HQ], mybir.ActivationFunctionType.Copy
    ).then_inc(s_act, 1)
    nc.scalar.activation(
        m5[:, HQ:], mb[:, HQ:], mybir.ActivationFunctionType.Copy
    ).then_inc(s_act, 1)

    # --- DVE: mask compare, grid adds, scale mult, mask mult halves
    H_OUT = W_OUT // 2
    nc.vector.wait_ge(s_cf, 16)
    nc.vector.wait_ge(s_cs, 16)  # thr bits
    nc.vector.tensor_scalar(tM.ap(), conf, thr, None, op.is_gt).then_inc(s_v, 1)
    nc.vector.wait_ge(s_x1, 16)
    nc.vector.tensor_tensor(
        P[:, :2, :, 1:3], xy[:, :2], a2[:, :2], op.add
    ).then_inc(s_v, 1)
    nc.vector.wait_ge(s_x2, 16)
    nc.vector.tensor_tensor(
        P[:, 2:, :, 1:3], xy[:, 2:], a2[:, 2:], op.add
    ).then_inc(s_v, 1)
    nc.vector.wait_ge(s_v, 3)
    nc.vector.wait_ge(s_act, 2)
    nc.vector.tensor_tensor(Pf, Pf, s5, op.mult).then_inc(s_v, 1)
    nc.vector.wait_ge(s_v, 4)
    nc.vector.wait_ge(s_act, 3)
    nc.vector.tensor_tensor(
        tP.ap()[:, :H_OUT], tP.ap()[:, :H_OUT], tM5.ap()[:, :H_OUT], op.mult
    ).then_inc(s_v, 1)
    nc.vector.wait_ge(s_v, 5)
    nc.vector.wait_ge(s_act, 4)
    nc.vector.tensor_tensor(
        tP.ap()[:, H_OUT:], tP.ap()[:, H_OUT:], tM5.ap()[:, H_OUT:], op.mult
    ).then_inc(s_v, 1)

    # --- output DMAs (one per half, both rings). The exit wait below rides
    # on the DVE chain: the last DMA's data lands ~1us into the ~7us NEFF
    # exit ritual, far before the runtime reads the output.
    nc.sync.wait_ge(s_v, 5)
    nc.sync.dma_start(OUT.ap()[:, :H_OUT], tP.ap()[:, :H_OUT]).then_inc(s_o, 16)
    nc.scalar.wait_ge(s_v, 6)
    nc.scalar.dma_start(OUT.ap()[:, H_OUT:], tP.ap()[:, H_OUT:]).then_inc(s_o, 16)

    # PE joins the exit barrier once compute retires
    nc.tensor.wait_ge(s_v, 6)
    nc.compile()
    return nc


def _pack_rows(heads_np):
    """Full head tensors -> [32, 128, Q, 5] padded AoS rows (pos, anchor, ch),
    heads concatenated in HEAD_ORDER."""
    blocks = []
    for h in HEAD_ORDER:
        arr = heads_np[h]
        hw = h * h
        sel = arr.reshape(B_TOTAL, 3, 85, hw)[:, :, 0:5, :]
        blocks.append(sel.transpose(0, 3, 1, 2).reshape(B_TOTAL, hw * 3, 5))
    rows = np.concatenate(blocks, axis=1)
    pad = np.zeros((B_TOTAL, ROWS_PAD - ROWS_VALID, 5), np.float32)
    rows = np.concatenate([rows, pad], axis=1)
    return rows.reshape(B_TOTAL, 128, Q, 5)


def _conf_f16_preserving(conf32, thr):
    """f16-encode conf so the device compare (f16 conf > f16 thr) equals the
    reference f32 compare elementwise: nudge any element whose rounding
    flipped the compare one ulp back across thr."""
    c16 = conf32.astype(np.float16)
    want = conf32 > thr
    for _ in range(3):
        got = c16.astype(np.float32) > thr
        bad = got != want
        if not bad.any():
            break
        target = np.where(want[bad], np.float16(np.inf), np.float16(-np.inf))
        c16[bad] = np.nextafter(c16[bad], target)
    return c16


def kernel(output_13, output_26, output_52, thresh):
    global _STATE
    if _STATE is None:
        _STATE = _build_program()
    nc = _STATE

    from concourse.bass_utils import run_bass_kernel_spmd

    heads_np = {13: np.asarray(output_13, np.float32),
                26: np.asarray(output_26, np.float32),
                52: np.asarray(output_52, np.float32)}
    thr = float(np.asarray(thresh))

    R = _pack_rows(heads_np)   # [32, 128, Q, 5]

    in_maps = []
    for core in range(N_CORES):
        sub = R[core * S:(core + 1) * S]          # [S, 128, Q, 5]
        din32 = sub[..., 1:3].transpose(1, 0, 2, 3).reshape(128, 2 * S * Q).copy()
        din16 = np.empty((128, W16), np.float16)
        din16[:, 0:C_WH] = _conf_f16_preserving(
            sub[..., 0].transpose(1, 0, 2).reshape(128, S * Q), thr
        )
        din16[:, C_WH:C_A2] = (
            sub[..., 3:5].transpose(1, 0, 2, 3).reshape(128, 2 * S * Q)
        )
        din16[:, C_A2:C_S5] = _A2
        din16[:, C_S5:C_BV] = _S5
        din16[:, C_BV:C_BV + S] = (core * S + np.arange(S, dtype=np.float32))[None, :]
        thr_bits = np.array([thr], np.float32).view(np.uint16)
        din16.view(np.uint16)[:, W16 - 2:W16] = thr_bits[None, :]
        in_maps.append({"din32": din32, "din16": din16})

    res = run_bass_kernel_spmd(nc, in_maps, core_ids=list(range(N_CORES)))

    # Unshard: per core, per section -> head blocks -> global row order.
    out = np.empty((ROWS_VALID * B_TOTAL, 5), np.float32)
    head_rows = {h: 3 * h * h for h in HEAD_ORDER}
    head_off = {}
    acc = 0
    for h in HEAD_ORDER:
        head_off[h] = acc
        acc += head_rows[h] * B_TOTAL
    for core in range(N_CORES):
        o = res.results[core]["dout"]             # [128, 1680] f16
        for s in range(S):
            bg = core * S + s
            sec = o[:, s * 5 * Q:(s + 1) * 5 * Q].astype(np.float32)
            rows = sec.reshape(128 * Q, 5)[:ROWS_VALID]
            off = 0
            for h in HEAD_ORDER:
                n = head_rows[h]
                dst = head_off[h] + bg * n
                out[dst:dst + n] = rows[off:off + n]
                off += n
    return out


# revision 47
# speedup vs baseline: 1.0611x; 1.0278x over previous
"""YOLOv3-style detection decode on 8 Trainium2 NeuronCores (pure batch data-parallel).

Contract: kernel(**inputs) takes the FULL inputs from setup_inputs() and returns
the FULL output of reference(). Batch dim 32 is sharded 4-per-core across 8
cores. Host ships only the 15 used channels, pre-packed into the output's AoS
row order; the device does the decode math (threshold mask, grid add, exp,
anchor scaling, batch-index fill).

Engine split (single chunk):
  GpSimd : P col0 := batch index b per section (4 memsets at entry, no DMA)
  ACT    : exp(w,h) into P cols 3:5
  DVE    : V0 m = (conf > thr) -> f16 mask [128,336]
           V1 grid-add (x,y f32 + A2) into P cols 1:3
           V2 P *= S5 (per-row scales (1,t,t,aw,ah), f16 2x mode)
           V3 P *= m broadcast 5-wide, in 4 quarters; each quarter's output
              DMA is issued as soon as it is ready (rings alternate)
Precision: x,y ride f32 (cancellation in col+x rules out f16); w,h ride f16
(exp output err ~ exponent abs err ~5e-4); conf rides f16 with a host-side
comparison-preserving rounding (elements whose f16 rounding would flip
`conf > thr` are nudged one ulp back, so the device f16 compare equals the
reference f32 compare exactly); output is f16 (max |box| ~27k < f16 max,
rel err ~5e-4 << 2e-2 tolerance).
"""
import sys

sys.path.insert(0, "/opt/trn_rl_repo")

import numpy as np

N_CORES = 8
B_TOTAL = 32
B_PER_CORE = B_TOTAL // N_CORES
IMG = 416.0

ANCHORS = {
    13: np.array([[116.0, 90.0], [156.0, 198.0], [373.0, 326.0]], np.float32),
    26: np.array([[30.0, 61.0], [62.0, 45.0], [59.0, 119.0]], np.float32),
    52: np.array([[10.0, 13.0], [16.0, 30.0], [33.0, 23.0]], np.float32),
}
HEAD_ORDER = [13, 26, 52]
ROWS_VALID = sum(3 * h * h for h in HEAD_ORDER)   # 10647
Q = 84                                            # row-slots per partition
ROWS_PAD = 128 * Q                                # 10752
S = B_PER_CORE                                    # 4 sections per core

# din32: [xy(672)]  f32
C_XY = 0
W32 = 2 * S * Q                                   # 672
# din16: [conf(336) | wh(672) | A2(168) | S5(420) | bvals(4) | negthr-f32(2)]
C_WH = S * Q                                      # 336
C_A2 = C_WH + 2 * S * Q                           # 1008
C_S5 = C_A2 + 2 * Q                               # 1176
C_BV = C_S5 + 5 * Q                               # 1596
W16 = C_BV + S + 2                                # 1602
W_OUT = 5 * S * Q                                 # 1680


def _build_constants():
    """Per-row-slot constants, indexed by flat row r = p*Q + q within a
    section: A2 (col,row grid offsets), S5 (1,t,t,aw,ah). Pad rows get 0."""
    r = np.arange(ROWS_PAD)
    a2 = np.zeros((ROWS_PAD, 2), np.float32)
    s5 = np.zeros((ROWS_PAD, 5), np.float32)
    off = 0
    for h in HEAD_ORDER:
        n = 3 * h * h
        lo, hi = off, off + n
        l = r[lo:hi] - off
        pos = l // 3
        anc = l % 3
        t = IMG / h
        a2[lo:hi, 0] = pos % h
        a2[lo:hi, 1] = pos // h
        s5[lo:hi, 0] = 1.0
        s5[lo:hi, 1] = t
        s5[lo:hi, 2] = t
        s5[lo:hi, 3] = ANCHORS[h][anc, 0]
        s5[lo:hi, 4] = ANCHORS[h][anc, 1]
        off = hi
    a2 = a2.reshape(128, Q, 2).reshape(128, 2 * Q)
    s5 = s5.reshape(128, Q, 5).reshape(128, 5 * Q)
    return a2.astype(np.float16), s5.astype(np.float16)


_A2, _S5 = _build_constants()

_STATE = None


def _build_program():
    """Raw Bacc program with manual semaphores (one chunk, V3/output in
    quarters so output DMA drains while DVE still computes)."""
    import concourse.bass as bass
    import concourse.bacc as bacc
    from concourse import mybir

    # Skip the Bass-constructor all-engine barrier (~0.8us).
    _orig_barrier = bass.Bass.all_engine_barrier
    bass.Bass.all_engine_barrier = lambda self, *a, **k: None
    try:
        nc = bacc.Bacc("TRN2", target_bir_lowering=False, debug=False)
    finally:
        bass.Bass.all_engine_barrier = _orig_barrier
    f32 = mybir.dt.float32
    f16 = mybir.dt.float16
    op = mybir.AluOpType

    IN32 = nc.dram_tensor("din32", [128, W32], f32, kind="ExternalInput")
    IN16 = nc.dram_tensor("din16", [128, W16], f16, kind="ExternalInput")
    OUT = nc.dram_tensor("dout", [128, W_OUT], f16, kind="ExternalOutput")

    t32 = nc.alloc_sbuf_tensor("t32", [128, W32], f32)
    t16 = nc.alloc_sbuf_tensor("t16", [128, W16], f16)
    tP = nc.alloc_sbuf_tensor("tp", [128, W_OUT], f16)
    tM = nc.alloc_sbuf_tensor("tm", [128, S * Q], f16)
    tM5 = nc.alloc_sbuf_tensor("tm5", [128, W_OUT], f16)

    s_cs = nc.alloc_semaphore("s_cs")   # A2+S5+bvals+thr DMA
    s_x1 = nc.alloc_semaphore("s_x1")   # xy sections 0-1 DMA
    s_x2 = nc.alloc_semaphore("s_x2")   # xy sections 2-3 DMA
    s_wc = nc.alloc_semaphore("s_wc")   # conf+wh DMA (one transfer)
    s_act = nc.alloc_semaphore("s_act")  # ACT exp(+1), M5a(+1), M5b(+1)
    s_v = nc.alloc_semaphore("s_v")     # DVE chain
    s_o = nc.alloc_semaphore("s_o")     # output DMAs

    thr = t16.ap()[:, W16 - 2:W16].bitcast(f32)
    bvals = (
        t16.ap()[:, C_BV:C_BV + S].unsqueeze(-1).broadcast_to((128, S, Q))
    )
    xy = t32.ap().rearrange("p (s t c) -> p s t c", s=S, c=2)
    conf = t16.ap()[:, 0:C_WH]
    wh = t16.ap()[:, C_WH:C_A2].rearrange("p (s t c) -> p s t c", s=S, c=2)
    a2 = (
        t16.ap()[:, C_A2:C_S5]
        .rearrange("p (t c) -> p t c", c=2)
        .unsqueeze(1)
        .broadcast_to((128, S, Q, 2))
    )
    s5 = t16.ap()[:, C_S5:C_BV].unsqueeze(1).broadcast_to((128, S, 5 * Q))
    P = tP.ap().rearrange("p (s t c) -> p s t c", s=S, c=5)
    Pf = tP.ap().rearrange("p (s f) -> p s f", s=S)

    # --- input DMAs; global land order ~ [conf+wh, A2S5, xy1, xy2]
    nc.sync.dma_start(t16.ap()[:, C_A2:], IN16.ap()[:, C_A2:]).then_inc(s_cs, 16)
    MIDXY = S * Q
    nc.sync.dma_start(t32.ap()[:, :MIDXY], IN32.ap()[:, :MIDXY]).then_inc(s_x1, 16)
    nc.sync.dma_start(t32.ap()[:, MIDXY:], IN32.ap()[:, MIDXY:]).then_inc(s_x2, 16)
    nc.scalar.dma_start(t16.ap()[:, :C_A2], IN16.ap()[:, :C_A2]).then_inc(s_wc, 16)

    # --- ACT: exp into P cols 3:5, then the contiguous 5-wide mask M5
    # (copy of the DVE-computed m, broadcast per row)
    nc.scalar.wait_ge(s_wc, 16)
    nc.scalar.activation(
        P[:, :, :, 3:5], wh, mybir.ActivationFunctionType.Exp, bias=0.0
    ).then_inc(s_act, 1)
    m5 = tM5.ap().rearrange("p (t c) -> p t c", c=5)
    mb = tM.ap().unsqueeze(-1).broadcast_to((128, S * Q, 5))
    HQ = S * Q // 2
    nc.scalar.wait_ge(s_v, 1)  # V0 done
    nc.scalar.activation(
        m5[:, :HQ], mb[:, :HQ], mybir.ActivationFunctionType.Copy
    ).then_inc(s_act, 1)
    nc.scalar.activation(
        m5[:, HQ:], mb[:, HQ:], mybir.ActivationFunctionType.Copy
    ).then_inc(s_act, 1)

    # --- DVE: mask compare, b-fill (in the idle window before xy lands),
    # grid adds, scale mult, mask mult halves
    H_OUT = W_OUT // 2
    nc.vector.wait_ge(s_wc, 16)
    nc.vector.wait_ge(s_cs, 16)  # thr bits
    nc.vector.tensor_scalar(tM.ap(), conf, thr, None, op.is_gt).then_inc(s_v, 1)
    nc.vector.tensor_copy(out=P[:, :, :, 0], in_=bvals).then_inc(s_v, 1)
    nc.vector.wait_ge(s_x1, 16)
    nc.vector.tensor_tensor(
        P[:, :2, :, 1:3], xy[:, :2], a2[:, :2], op.add
    ).then_inc(s_v, 1)
    nc.vector.wait_ge(s_x2, 16)
    nc.vector.tensor_tensor(
        P[:, 2:, :, 1:3], xy[:, 2:], a2[:, 2:], op.add
    ).then_inc(s_v, 1)
    nc.vector.wait_ge(s_v, 4)
    nc.vector.wait_ge(s_act, 1)
    nc.vector.tensor_tensor(Pf, Pf, s5, op.mult).then_inc(s_v, 1)
    nc.vector.wait_ge(s_v, 5)
    nc.vector.wait_ge(s_act, 2)
    nc.vector.tensor_tensor(
        tP.ap()[:, :H_OUT], tP.ap()[:, :H_OUT], tM5.ap()[:, :H_OUT], op.mult
    ).then_inc(s_v, 1)
    nc.vector.wait_ge(s_v, 6)
    nc.vector.wait_ge(s_act, 3)
    nc.vector.tensor_tensor(
        tP.ap()[:, H_OUT:], tP.ap()[:, H_OUT:], tM5.ap()[:, H_OUT:], op.mult
    ).then_inc(s_v, 1)

    # --- output DMAs (one per half, both rings). The exit wait below rides
    # on the DVE chain: the last DMA's data lands ~1us into the ~7us NEFF
    # exit ritual, far before the runtime reads the output.
    nc.sync.wait_ge(s_v, 6)
    nc.sync.dma_start(OUT.ap()[:, :H_OUT], tP.ap()[:, :H_OUT]).then_inc(s_o, 16)
    nc.scalar.wait_ge(s_v, 7)
    nc.scalar.dma_start(OUT.ap()[:, H_OUT:], tP.ap()[:, H_OUT:]).then_inc(s_o, 16)

    # PE joins the exit barrier once compute retires
    nc.tensor.wait_ge(s_v, 7)
    nc.compile()
    return nc


def _pack_rows(heads_np):
    """Full head tensors -> [32, 128, Q, 5] padded AoS rows (pos, anchor, ch),
    heads concatenated in HEAD_ORDER."""
    blocks = []
    for h in HEAD_ORDER:
        arr = heads_np[h]
        hw = h * h
        sel = arr.reshape(B_TOTAL, 3, 85, hw)[:, :, 0:5, :]
        blocks.append(sel.transpose(0, 3, 1, 2).reshape(B_TOTAL, hw * 3, 5))
    rows = np.concatenate(blocks, axis=1)
    pad = np.zeros((B_TOTAL, ROWS_PAD - ROWS_VALID, 5), np.float32)
    rows = np.concatenate([rows, pad], axis=1)
    return rows.reshape(B_TOTAL, 128, Q, 5)


def _conf_f16_preserving(conf32, thr):
    """f16-encode conf so the device compare (f16 conf > f16 thr) equals the
    reference f32 compare elementwise: nudge any element whose rounding
    flipped the compare one ulp back across thr."""
    c16 = conf32.astype(np.float16)
    want = conf32 > thr
    for _ in range(3):
        got = c16.astype(np.float32) > thr
        bad = got != want
        if not bad.any():
            break
        target = np.where(want[bad], np.float16(np.inf), np.float16(-np.inf))
        c16[bad] = np.nextafter(c16[bad], target)
    return c16


def kernel(output_13, output_26, output_52, thresh):
    global _STATE
    if _STATE is None:
        _STATE = _build_program()
    nc = _STATE

    from concourse.bass_utils import run_bass_kernel_spmd

    heads_np = {13: np.asarray(output_13, np.float32),
                26: np.asarray(output_26, np.float32),
                52: np.asarray(output_52, np.float32)}
    thr = float(np.asarray(thresh))

    R = _pack_rows(heads_np)   # [32, 128, Q, 5]

    in_maps = []
    for core in range(N_CORES):
        sub = R[core * S:(core + 1) * S]          # [S, 128, Q, 5]
        din32 = sub[..., 1:3].transpose(1, 0, 2, 3).reshape(128, 2 * S * Q).copy()
        din16 = np.empty((128, W16), np.float16)
        din16[:, 0:C_WH] = _conf_f16_preserving(
            sub[..., 0].transpose(1, 0, 2).reshape(128, S * Q), thr
        )
        din16[:, C_WH:C_A2] = (
            sub[..., 3:5].transpose(1, 0, 2, 3).reshape(128, 2 * S * Q)
        )
        din16[:, C_A2:C_S5] = _A2
        din16[:, C_S5:C_BV] = _S5
        din16[:, C_BV:C_BV + S] = (core * S + np.arange(S, dtype=np.float32))[None, :]
        thr_bits = np.array([thr], np.float32).view(np.uint16)
        din16.view(np.uint16)[:, W16 - 2:W16] = thr_bits[None, :]
        in_maps.append({"din32": din32, "din16": din16})

    res = run_bass_kernel_spmd(nc, in_maps, core_ids=list(range(N_CORES)))

    # Unshard: per core, per section -> head blocks -> global row order.
    out = np.empty((ROWS_VALID * B_TOTAL, 5), np.float32)
    head_rows = {h: 3 * h * h for h in HEAD_ORDER}
    head_off = {}
    acc = 0
    for h in HEAD_ORDER:
        head_off[h] = acc
        acc += head_rows[h] * B_TOTAL
    for core in range(N_CORES):
        o = res.results[core]["dout"]             # [128, 1680] f16
        for s in range(S):
            bg = core * S + s
            sec = o[:, s * 5 * Q:(s + 1) * 5 * Q].astype(np.float32)
            rows = sec.reshape(128 * Q, 5)[:ROWS_VALID]
            off = 0
            for h in HEAD_ORDER:
                n = head_rows[h]
                dst = head_off[h] + bg * n
                out[dst:dst + n] = rows[off:off + n]
                off += n
    return out


# revision 48
# speedup vs baseline: 1.0695x; 1.0079x over previous
"""YOLOv3-style detection decode on 8 Trainium2 NeuronCores (pure batch data-parallel).

Contract: kernel(**inputs) takes the FULL inputs from setup_inputs() and returns
the FULL output of reference(). Batch dim 32 is sharded 4-per-core across 8
cores. Host ships only the 15 used channels, pre-packed into the output's AoS
row order; the device does the decode math (threshold mask, grid add, exp,
anchor scaling, batch-index fill).

Engine split (single chunk):
  GpSimd : P col0 := batch index b per section (4 memsets at entry, no DMA)
  ACT    : exp(w,h) into P cols 3:5
  DVE    : V0 m = (conf > thr) -> f16 mask [128,336]
           V1 grid-add (x,y f32 + A2) into P cols 1:3
           V2 P *= S5 (per-row scales (1,t,t,aw,ah), f16 2x mode)
           V3 P *= m broadcast 5-wide, in 4 quarters; each quarter's output
              DMA is issued as soon as it is ready (rings alternate)
Precision: x,y ride f32 (cancellation in col+x rules out f16); w,h ride f16
(exp output err ~ exponent abs err ~5e-4); conf rides f16 with a host-side
comparison-preserving rounding (elements whose f16 rounding would flip
`conf > thr` are nudged one ulp back, so the device f16 compare equals the
reference f32 compare exactly); output is f16 (max |box| ~27k < f16 max,
rel err ~5e-4 << 2e-2 tolerance).
"""
import sys

sys.path.insert(0, "/opt/trn_rl_repo")

import numpy as np

N_CORES = 8
B_TOTAL = 32
B_PER_CORE = B_TOTAL // N_CORES
IMG = 416.0

ANCHORS = {
    13: np.array([[116.0, 90.0], [156.0, 198.0], [373.0, 326.0]], np.float32),
    26: np.array([[30.0, 61.0], [62.0, 45.0], [59.0, 119.0]], np.float32),
    52: np.array([[10.0, 13.0], [16.0, 30.0], [33.0, 23.0]], np.float32),
}
HEAD_ORDER = [13, 26, 52]
ROWS_VALID = sum(3 * h * h for h in HEAD_ORDER)   # 10647
Q = 84                                            # row-slots per partition
ROWS_PAD = 128 * Q                                # 10752
S = B_PER_CORE                                    # 4 sections per core

# din32: [xy(672)]  f32
C_XY = 0
W32 = 2 * S * Q                                   # 672
# din16: [conf(336) | wh(672) | A2(168) | S5(420) | bvals(4) | negthr-f32(2)]
C_WH = S * Q                                      # 336
C_A2 = C_WH + 2 * S * Q                           # 1008
C_S5 = C_A2 + 2 * Q                               # 1176
C_BV = C_S5 + 5 * Q                               # 1596
W16 = C_BV + S + 2                                # 1602
W_OUT = 5 * S * Q                                 # 1680


def _build_constants():
    """Per-row-slot constants, indexed by flat row r = p*Q + q within a
    section: A2 (col,row grid offsets), S5 (1,t,t,aw,ah). Pad rows get 0."""
    r = np.arange(ROWS_PAD)
    a2 = np.zeros((ROWS_PAD, 2), np.float32)
    s5 = np.zeros((ROWS_PAD, 5), np.float32)
    off = 0
    for h in HEAD_ORDER:
        n = 3 * h * h
        lo, hi = off, off + n
        l = r[lo:hi] - off
        pos = l // 3
        anc = l % 3
        t = IMG / h
        a2[lo:hi, 0] = pos % h
        a2[lo:hi, 1] = pos // h
        s5[lo:hi, 0] = 1.0
        s5[lo:hi, 1] = t
        s5[lo:hi, 2] = t
        s5[lo:hi, 3] = ANCHORS[h][anc, 0]
        s5[lo:hi, 4] = ANCHORS[h][anc, 1]
        off = hi
    a2 = a2.reshape(128, Q, 2).reshape(128, 2 * Q)
    s5 = s5.reshape(128, Q, 5).reshape(128, 5 * Q)
    return a2.astype(np.float16), s5.astype(np.float16)


_A2, _S5 = _build_constants()

_STATE = None


def _build_program():
    """Raw Bacc program with manual semaphores (one chunk, V3/output in
    quarters so output DMA drains while DVE still computes)."""
    import concourse.bass as bass
    import concourse.bacc as bacc
    from concourse import mybir

    # Skip the Bass-constructor all-engine barrier (~0.8us).
    _orig_barrier = bass.Bass.all_engine_barrier
    bass.Bass.all_engine_barrier = lambda self, *a, **k: None
    try:
        nc = bacc.Bacc("TRN2", target_bir_lowering=False, debug=False)
    finally:
        bass.Bass.all_engine_barrier = _orig_barrier
    f32 = mybir.dt.float32
    f16 = mybir.dt.float16
    op = mybir.AluOpType

    IN32 = nc.dram_tensor("din32", [128, W32], f32, kind="ExternalInput")
    IN16 = nc.dram_tensor("din16", [128, W16], f16, kind="ExternalInput")
    OUT = nc.dram_tensor("dout", [128, W_OUT], f16, kind="ExternalOutput")

    t32 = nc.alloc_sbuf_tensor("t32", [128, W32], f32)
    t16 = nc.alloc_sbuf_tensor("t16", [128, W16], f16)
    tP = nc.alloc_sbuf_tensor("tp", [128, W_OUT], f16)
    tM = nc.alloc_sbuf_tensor("tm", [128, S * Q], f16)
    tM5 = nc.alloc_sbuf_tensor("tm5", [128, W_OUT], f16)

    s_cs = nc.alloc_semaphore("s_cs")   # A2+S5+bvals+thr DMA
    s_x1 = nc.alloc_semaphore("s_x1")   # xy sections 0-1 DMA
    s_x2 = nc.alloc_semaphore("s_x2")   # xy sections 2-3 DMA
    s_wc = nc.alloc_semaphore("s_wc")   # conf+wh DMA (one transfer)
    s_act = nc.alloc_semaphore("s_act")  # ACT exp(+1), M5a(+1), M5b(+1)
    s_v = nc.alloc_semaphore("s_v")     # DVE chain
    s_o = nc.alloc_semaphore("s_o")     # output DMAs

    thr = t16.ap()[:, W16 - 2:W16].bitcast(f32)
    bvals = (
        t16.ap()[:, C_BV:C_BV + S].unsqueeze(-1).broadcast_to((128, S, Q))
    )
    xy = t32.ap().rearrange("p (s t c) -> p s t c", s=S, c=2)
    conf = t16.ap()[:, 0:C_WH]
    wh = t16.ap()[:, C_WH:C_A2].rearrange("p (s t c) -> p s t c", s=S, c=2)
    a2 = (
        t16.ap()[:, C_A2:C_S5]
        .rearrange("p (t c) -> p t c", c=2)
        .unsqueeze(1)
        .broadcast_to((128, S, Q, 2))
    )
    s5 = t16.ap()[:, C_S5:C_BV].unsqueeze(1).broadcast_to((128, S, 5 * Q))
    P = tP.ap().rearrange("p (s t c) -> p s t c", s=S, c=5)
    Pf = tP.ap().rearrange("p (s f) -> p s f", s=S)

    # --- input DMAs; rings balanced 322KB/430KB, global land order
    # ~ [A2S5 ~ conf+wh, xy1, xy2]
    nc.sync.dma_start(t16.ap()[:, C_A2:], IN16.ap()[:, C_A2:]).then_inc(s_cs, 16)
    MIDXY = S * Q
    nc.sync.dma_start(t32.ap()[:, :MIDXY], IN32.ap()[:, :MIDXY]).then_inc(s_x1, 16)
    nc.scalar.dma_start(t16.ap()[:, :C_A2], IN16.ap()[:, :C_A2]).then_inc(s_wc, 16)
    nc.scalar.dma_start(t32.ap()[:, MIDXY:], IN32.ap()[:, MIDXY:]).then_inc(s_x2, 16)

    # --- ACT: exp into P cols 3:5, then the contiguous 5-wide mask M5
    # (copy of the DVE-computed m, broadcast per row)
    nc.scalar.wait_ge(s_wc, 16)
    nc.scalar.activation(
        P[:, :, :, 3:5], wh, mybir.ActivationFunctionType.Exp, bias=0.0
    ).then_inc(s_act, 1)
    m5 = tM5.ap().rearrange("p (t c) -> p t c", c=5)
    mb = tM.ap().unsqueeze(-1).broadcast_to((128, S * Q, 5))
    HQ = S * Q // 2
    nc.scalar.wait_ge(s_v, 1)  # V0 done
    nc.scalar.activation(
        m5[:, :HQ], mb[:, :HQ], mybir.ActivationFunctionType.Copy
    ).then_inc(s_act, 1)
    nc.scalar.activation(
        m5[:, HQ:], mb[:, HQ:], mybir.ActivationFunctionType.Copy
    ).then_inc(s_act, 1)

    # --- DVE: mask compare, b-fill (in the idle window before xy lands),
    # grid adds, scale mult, mask mult halves
    H_OUT = W_OUT // 2
    nc.vector.wait_ge(s_wc, 16)
    nc.vector.wait_ge(s_cs, 16)  # thr bits
    nc.vector.tensor_scalar(tM.ap(), conf, thr, None, op.is_gt).then_inc(s_v, 1)
    nc.vector.tensor_copy(out=P[:, :, :, 0], in_=bvals).then_inc(s_v, 1)
    nc.vector.wait_ge(s_x1, 16)
    nc.vector.tensor_tensor(
        P[:, :2, :, 1:3], xy[:, :2], a2[:, :2], op.add
    ).then_inc(s_v, 1)
    nc.vector.wait_ge(s_x2, 16)
    nc.vector.tensor_tensor(
        P[:, 2:, :, 1:3], xy[:, 2:], a2[:, 2:], op.add
    ).then_inc(s_v, 1)
    nc.vector.wait_ge(s_v, 4)
    nc.vector.wait_ge(s_act, 1)
    nc.vector.tensor_tensor(Pf, Pf, s5, op.mult).then_inc(s_v, 1)
    nc.vector.wait_ge(s_v, 5)
    nc.vector.wait_ge(s_act, 2)
    nc.vector.tensor_tensor(
        tP.ap()[:, :H_OUT], tP.ap()[:, :H_OUT], tM5.ap()[:, :H_OUT], op.mult
    ).then_inc(s_v, 1)
    nc.vector.wait_ge(s_v, 6)
    nc.vector.wait_ge(s_act, 3)
    nc.vector.tensor_tensor(
        tP.ap()[:, H_OUT:], tP.ap()[:, H_OUT:], tM5.ap()[:, H_OUT:], op.mult
    ).then_inc(s_v, 1)

    # --- output DMAs (one per half, both rings). The exit wait below rides
    # on the DVE chain: the last DMA's data lands ~1us into the ~7us NEFF
    # exit ritual, far before the runtime reads the output.
    nc.sync.wait_ge(s_v, 6)
    nc.sync.dma_start(OUT.ap()[:, :H_OUT], tP.ap()[:, :H_OUT]).then_inc(s_o, 16)
    nc.scalar.wait_ge(s_v, 7)
    nc.scalar.dma_start(OUT.ap()[:, H_OUT:], tP.ap()[:, H_OUT:]).then_inc(s_o, 16)

    # PE joins the exit barrier once compute retires
    nc.tensor.wait_ge(s_v, 7)
    nc.compile()
    return nc


def _pack_rows(heads_np):
    """Full head tensors -> [32, 128, Q, 5] padded AoS rows (pos, anchor, ch),
    heads concatenated in HEAD_ORDER."""
    blocks = []
    for h in HEAD_ORDER:
        arr = heads_np[h]
        hw = h * h
        sel = arr.reshape(B_TOTAL, 3, 85, hw)[:, :, 0:5, :]
        blocks.append(sel.transpose(0, 3, 1, 2).reshape(B_TOTAL, hw * 3, 5))
    rows = np.concatenate(blocks, axis=1)
    pad = np.zeros((B_TOTAL, ROWS_PAD - ROWS_VALID, 5), np.float32)
    rows = np.concatenate([rows, pad], axis=1)
    return rows.reshape(B_TOTAL, 128, Q, 5)


def _conf_f16_preserving(conf32, thr):
    """f16-encode conf so the device compare (f16 conf > f16 thr) equals the
    reference f32 compare elementwise: nudge any element whose rounding
    flipped the compare one ulp back across thr."""
    c16 = conf32.astype(np.float16)
    want = conf32 > thr
    for _ in range(3):
        got = c16.astype(np.float32) > thr
        bad = got != want
        if not bad.any():
            break
        target = np.where(want[bad], np.float16(np.inf), np.float16(-np.inf))
        c16[bad] = np.nextafter(c16[bad], target)
    return c16


def kernel(output_13, output_26, output_52, thresh):
    global _STATE
    if _STATE is None:
        _STATE = _build_program()
    nc = _STATE

    from concourse.bass_utils import run_bass_kernel_spmd

    heads_np = {13: np.asarray(output_13, np.float32),
                26: np.asarray(output_26, np.float32),
                52: np.asarray(output_52, np.float32)}
    thr = float(np.asarray(thresh))

    R = _pack_rows(heads_np)   # [32, 128, Q, 5]

    in_maps = []
    for core in range(N_CORES):
        sub = R[core * S:(core + 1) * S]          # [S, 128, Q, 5]
        din32 = sub[..., 1:3].transpose(1, 0, 2, 3).reshape(128, 2 * S * Q).copy()
        din16 = np.empty((128, W16), np.float16)
        din16[:, 0:C_WH] = _conf_f16_preserving(
            sub[..., 0].transpose(1, 0, 2).reshape(128, S * Q), thr
        )
        din16[:, C_WH:C_A2] = (
            sub[..., 3:5].transpose(1, 0, 2, 3).reshape(128, 2 * S * Q)
        )
        din16[:, C_A2:C_S5] = _A2
        din16[:, C_S5:C_BV] = _S5
        din16[:, C_BV:C_BV + S] = (core * S + np.arange(S, dtype=np.float32))[None, :]
        thr_bits = np.array([thr], np.float32).view(np.uint16)
        din16.view(np.uint16)[:, W16 - 2:W16] = thr_bits[None, :]
        in_maps.append({"din32": din32, "din16": din16})

    res = run_bass_kernel_spmd(nc, in_maps, core_ids=list(range(N_CORES)))

    # Unshard: per core, per section -> head blocks -> global row order.
    out = np.empty((ROWS_VALID * B_TOTAL, 5), np.float32)
    head_rows = {h: 3 * h * h for h in HEAD_ORDER}
    head_off = {}
    acc = 0
    for h in HEAD_ORDER:
        head_off[h] = acc
        acc += head_rows[h] * B_TOTAL
    for core in range(N_CORES):
        o = res.results[core]["dout"]             # [128, 1680] f16
        for s in range(S):
            bg = core * S + s
            sec = o[:, s * 5 * Q:(s + 1) * 5 * Q].astype(np.float32)
            rows = sec.reshape(128 * Q, 5)[:ROWS_VALID]
            off = 0
            for h in HEAD_ORDER:
                n = head_rows[h]
                dst = head_off[h] + bg * n
                out[dst:dst + n] = rows[off:off + n]
                off += n
    return out
